# revision 32
# baseline (speedup 1.0000x reference)
"""GATv2-style DeepGraphConvLayer on 8 Trainium2 NeuronCores.

Strategy (graph-data-parallel, per sharding hint):
  - 8 cores x 64 graphs each; each graph = 128 nodes / 1024 edges.
  - All gather/scatter (segment ops) done as PE matmuls against host-built
    one-hot matrices (bf16, exact 0/1/2 values):
      * ehT   = feat.T-gathered-sum   via lhsT=feat chunk, rhs=Asum[n,e]
      * fs    = feat[src] (edge-row)  via lhsT=Msrc[n,e] chunk, rhs=feat
      * s     = segsum(expl) by dst   via lhsT=MdstT[e,n] chunk, rhs=expl_er
      * rs_er = recip_s[dst] gather   via lhsT=Mdst[n,e] chunk, rhs=recip_s
      * rst   = segsum(wmsg) by dst   via lhsT=MdstT[e,n] chunk, rhs=wmsg
  - leaky_relu fused into the PSUM->SBUF evacuation (ACT Lrelu).
  - softmax denominators via segment-sum of exp (no max subtraction: logits
    are O(1); shift cancels mathematically anyway).
  - messages weighted by normalized a at the edge level; BatchNorm batch
  - stats all-reduced across the 8 cores with a DRAM collective.
Outputs: out [N,256] f32, attn written edge-major [g,1024,8] and transposed
to [B,H,EPG] on the host during unshard (pure data movement).
"""
import numpy as np
import ml_dtypes
import concourse.bass as bass
import concourse.bacc as bacc
import concourse.tile as tile
from concourse import mybir
from concourse.bass_utils import run_bass_kernel_spmd
from concourse.masks import make_identity

F32 = mybir.dt.float32
BF16 = mybir.dt.bfloat16

NCORES = 8
B = 512
NODE = 128
EPG = 1024
H = 8
D = 32
IN = 256
F = 256           # H*D
G = B // NCORES   # 64 graphs per core
NPC = G * NODE    # 8192 nodes per core
EPC = G * EPG     # 65536 edges per core
NEG_SLOPE = 0.2
BN_EPS = 1e-5
NTOT = B * NODE   # 65536

# bf16 blob layout (per graph), columns along free dim:
#   Asum  [128, 1024]  cols 0:1024      (n-part, e)
#   Msrc  [128, 1024]  cols 1024:2048   (n-part, e)  chunks: [:, 128c:128c+128]
#   MdstT [128, 1024]  cols 2048:3072   (e-part chunks, n): chunk c at [:, 128c:]
#   Mdst  [128, 1024]  cols 3072:4096   (n-part, e)
#   xT    [128, 256]   cols 4096:4352   (in-part chunks, n): chunk k at [:, 128k:]
BLOB_COLS = 4352
A_OFF, MS_OFF, MDT_OFF, MD_OFF, XT_OFF = 0, 1024, 2048, 3072, 4096


def build_bass(use_collective=True, debug_dump=False):
    nc = bacc.Bacc(None, target_bir_lowering=False, debug=False)

    blob = nc.dram_tensor("blob", [G, 128, BLOB_COLS], BF16, kind="ExternalInput")
    xf = nc.dram_tensor("xf", [G, 128, F], F32, kind="ExternalInput")
    fcwT = nc.dram_tensor("fcwT", [128, 2, F], BF16, kind="ExternalInput")
    fcb = nc.dram_tensor("fcb", [1, F], F32, kind="ExternalInput")
    wb = nc.dram_tensor("wb", [128, 2, H], BF16, kind="ExternalInput")
    wcomb = nc.dram_tensor("wcomb", [128, 2, H], BF16, kind="ExternalInput")
    dwb = nc.dram_tensor("dwb", [1, H], BF16, kind="ExternalInput")
    gam = nc.dram_tensor("gam", [1, F], F32, kind="ExternalInput")
    bet = nc.dram_tensor("bet", [1, F], F32, kind="ExternalInput")

    out = nc.dram_tensor("out", [G, 128, F], F32, kind="ExternalOutput")
    attn = nc.dram_tensor("attn", [G, 128, 8, H], F32, kind="ExternalOutput")

    dbg = None
    if debug_dump:
        dbg = nc.dram_tensor("dbg", [6, 128, 1024], F32, kind="ExternalOutput")
    cc_in = nc.dram_tensor("cc_in", [1, 512], F32)
    bn_dram = nc.dram_tensor("bn_dram", [2, F], F32)
    cc_out = nc.dram_tensor("cc_out", [1, 512], F32)

    with nc.allow_low_precision(reason="bf16 edge pipeline by design"), \
         tile.TileContext(nc) as tc:
        with (
            tc.tile_pool(name="statics", bufs=1) as statics,
            tc.tile_pool(name="rstbuf", bufs=1) as rstbuf,
            tc.tile_pool(name="blobs", bufs=5) as blobs,
            tc.tile_pool(name="work", bufs=3) as work,
            tc.tile_pool(name="bnp", bufs=2) as bnp,
            tc.tile_pool(name="small", bufs=2) as small,
            tc.tile_pool(name="ps_big", bufs=2, space="PSUM") as ps_big,
            tc.tile_pool(name="ps_log", bufs=1, space="PSUM") as ps_log,
            tc.tile_pool(name="ps_small", bufs=1, space="PSUM") as ps_small,
            tc.tile_pool(name="ps_fs", bufs=2, space="PSUM") as ps_fs,
            tc.tile_pool(name="ps_rst", bufs=1, space="PSUM") as ps_rst,
            tc.tile_pool(name="ps_bn", bufs=1, space="PSUM") as ps_bn,
        ):
            # ---------------- statics ----------------
            ident = statics.tile([128, 128], F32)
            make_identity(nc, ident[:])
            ones_bf = statics.tile([128, 1], BF16)
            nc.vector.memset(ones_bf[:], 1.0)
            ones_row = statics.tile([1, 128], BF16)
            nc.vector.memset(ones_row[:], 1.0)
            fcwT_t = statics.tile([128, 2, F], BF16)
            nc.gpsimd.dma_start(fcwT_t[:], fcwT[:])
            wb_t = statics.tile([128, 2, H], BF16)
            nc.gpsimd.dma_start(wb_t[:], wb[:])
            wcomb_t = statics.tile([128, 2, H], BF16)
            nc.gpsimd.dma_start(wcomb_t[:], wcomb[:])
            dwb_t = statics.tile([1, H], BF16)
            nc.gpsimd.dma_start(dwb_t[:], dwb[:])
            fcb_t = statics.tile([1, F], F32)
            nc.gpsimd.dma_start(fcb_t[:], fcb[:])
            gam_t = statics.tile([1, F], F32)
            nc.gpsimd.dma_start(gam_t[:], gam[:])
            bet_t = statics.tile([1, F], F32)
            nc.gpsimd.dma_start(bet_t[:], bet[:])
            fcb_rep = statics.tile([128, F], F32)
            nc.gpsimd.dma_start(
                out=fcb_rep[:],
                in_=bass.AP(tensor=fcb[:].tensor, offset=fcb[:].offset,
                            ap=[[0, 128], [1, F]]),
            )

            # rst for all 64 graphs stays in SBUF: [128, 64*256] f32
            rst_all = rstbuf.tile([128, G * F], F32)

            bnacc_ps = ps_bn.tile([1, 512], F32)
            bn_ps = bnacc_ps[:, :256]
            bnsq_ps = bnacc_ps[:, 256:]

            # "touch" matmuls so statics' DMA waits are absorbed by PE before
            # the real matmuls (Matmult carries at most ONE sem wait).
            nc.tensor.matmul(bn_ps[:1, :1], fcwT_t[:1, 0, :1],
                             fcwT_t[:1, 0, :1], start=True, stop=True)
            nc.tensor.matmul(bn_ps[:1, :1], wb_t[:1, 0, :1],
                             wb_t[:1, 0, :1], start=True, stop=True)
            nc.tensor.matmul(bn_ps[:1, :1], ones_bf[:1, :1],
                             ones_bf[:1, :1], start=True, stop=True)
            nc.tensor.matmul(bn_ps[:1, :1], wcomb_t[:1, 0, :1],
                             wcomb_t[:1, 0, :1], start=True, stop=True)
            nc.tensor.matmul(bn_ps[:1, :1], ones_row[:1, :1],
                             dwb_t[:1, :1], start=True, stop=True)
            nc.tensor.transpose(bn_ps[:1, :1], ident[:1, :1], ident[:1, :1])

            for g in range(G):
                # -------- per-graph loads (2 DMAs) --------
                bl = blobs.tile([128, BLOB_COLS], BF16, tag="bl")
                nc.sync.dma_start(bl[:], blob[g])
                xg = blobs.tile([128, F], F32, tag="xg")
                nc.sync.dma_start(xg[:], xf[g])

                # -------- fc: feat = x @ fc_w.T + b --------
                feat_ps = ps_big.tile([128, 512], F32, tag="big")
                for k in range(2):
                    nc.tensor.matmul(
                        feat_ps[:, :F],
                        bl[:, XT_OFF + 128 * k:XT_OFF + 128 * (k + 1)],
                        fcwT_t[:, k, :],
                        start=(k == 0), stop=(k == 1),
                    )
                for k in range(2):
                    nc.tensor.matmul(
                        feat_ps[:, 256:256 + H],
                        bl[:, XT_OFF + 128 * k:XT_OFF + 128 * (k + 1)],
                        wcomb_t[:, k, :],
                        start=(k == 0), stop=False,
                    )
                nc.tensor.matmul(
                    feat_ps[:, 256:256 + H], ones_row[:], dwb_t[:],
                    start=False, stop=True,
                )
                feat_bf = work.tile([128, F], BF16, tag="feat")
                nc.vector.tensor_tensor(
                    out=feat_bf[:], in0=feat_ps[:, :F], in1=fcb_rep[:],
                    op=mybir.AluOpType.add,
                )
                dw_bf = small.tile([128, H], BF16, tag="dw_bf")
                nc.vector.tensor_copy(dw_bf[:], feat_ps[:, 256:256 + H])

                # -------- ehT + leaky (2 fchunks x 2 ehalves) --------
                lky0 = work.tile([128, 1024], BF16, tag="lky0")
                lky1 = work.tile([128, 1024], BF16, tag="lky1")
                lky = [lky0, lky1]
                for k in range(2):
                    for h2 in range(2):
                        eh_ps = ps_big.tile([128, 512], F32, tag="big")
                        nc.tensor.matmul(
                            eh_ps[:],
                            feat_bf[:, 128 * k:128 * (k + 1)],
                            bl[:, A_OFF + 512 * h2:A_OFF + 512 * (h2 + 1)],
                            start=True, stop=True,
                        )
                        nc.scalar.activation(
                            lky[k][:, 512 * h2:512 * (h2 + 1)], eh_ps[:],
                            mybir.ActivationFunctionType.Relu, scale=0.8,
                        )

                if debug_dump and g == 0:
                    dfeat = work.tile([128, F], F32, tag="dfeat")
                    nc.vector.tensor_copy(dfeat[:], feat_bf[:])
                    nc.sync.dma_start(dbg[0, :, :F], dfeat[:])
                    dlky = work.tile([128, 1024], F32, tag="dlky")
                    nc.vector.tensor_copy(dlky[:], lky[0][:])
                    nc.sync.dma_start(dbg[1], dlky[:])

                # -------- logits + exp --------
                expl_T = small.tile([8, 1024], F32, tag="explT")
                for h2 in range(2):
                    log_ps = ps_log.tile([8, 512], F32, tag="log")
                    for k in range(2):
                        nc.tensor.matmul(
                            log_ps[:],
                            wb_t[:, k, :],
                            lky[k][:, 512 * h2:512 * (h2 + 1)],
                            start=(k == 0), stop=False,
                        )
                    nc.tensor.matmul(
                        log_ps[:],
                        dw_bf[:],
                        bl[:, A_OFF + 512 * h2:A_OFF + 512 * (h2 + 1)],
                        start=False, stop=True,
                    )
                    nc.scalar.activation(
                        expl_T[:, 512 * h2:512 * (h2 + 1)], log_ps[:],
                        mybir.ActivationFunctionType.Exp,
                    )

                if debug_dump and g == 0:
                    nc.sync.dma_start(dbg[2, :8, :], expl_T[:])

                # -------- expl_er: [8,1024] -> edge-row [128, 8c+h] --------
                er_ps = ps_small.tile([128, 512], F32, tag="small")
                for c in range(8):
                    nc.tensor.transpose(
                        er_ps[:, 8 * c:8 * (c + 1)],
                        expl_T[:, 128 * c:128 * (c + 1)],
                        ident[:8, :8],
                    )
                expl_er = small.tile([128, 64], F32, tag="expl_er")
                nc.vector.tensor_copy(expl_er[:], er_ps[:, :64])
                expl_er_bf = small.tile([128, 64], BF16, tag="expl_er_bf")
                nc.vector.tensor_copy(expl_er_bf[:], expl_er[:])

                # -------- s = segsum(expl) by dst; recip --------
                for c in range(8):
                    nc.tensor.matmul(
                        er_ps[:, 64:64 + 8],
                        bl[:, MDT_OFF + 128 * c:MDT_OFF + 128 * (c + 1)],
                        expl_er_bf[:, 8 * c:8 * (c + 1)],
                        start=(c == 0), stop=(c == 7),
                    )
                # clamp: isolated nodes have s=0; recip(0)=inf would
                # poison the gather matmul (0*inf=NaN)
                s_cl = small.tile([128, 8], F32, tag="s_cl")
                nc.vector.tensor_scalar_max(s_cl[:], er_ps[:, 64:64 + 8], 1e-12)
                recip_bf = small.tile([128, 8], BF16, tag="recip")
                nc.vector.reciprocal(recip_bf[:], s_cl[:])

                # -------- rs_er = recip_s[dst] gather (edge-row) --------
                for c in range(8):
                    nc.tensor.matmul(
                        er_ps[:, 128 + 8 * c:128 + 8 * (c + 1)],
                        bl[:, MD_OFF + 128 * c:MD_OFF + 128 * (c + 1)],
                        recip_bf[:],
                        start=True, stop=True,
                    )
                if debug_dump and g == 0:
                    dser = work.tile([128, 192], F32, tag="dser")
                    nc.vector.tensor_copy(dser[:], er_ps[:, :192])
                    nc.sync.dma_start(dbg[3, :, :192], dser[:])

                a_er = small.tile([128, 64], F32, tag="a_er")
                nc.vector.tensor_tensor(
                    out=a_er[:], in0=expl_er[:], in1=er_ps[:, 128:128 + 64],
                    op=mybir.AluOpType.mult,
                )
                # attn out in a_er layout [128 e', (c, h)]; host reorders
                nc.sync.dma_start(attn[g], a_er[:].rearrange(
                    "p (c h) -> p c h", c=8))

                if debug_dump and g == 0:
                    nc.sync.dma_start(dbg[4, :, :64], a_er[:])

                # -------- fs + wmsg (2 chunks per psum tile) --------
                wmsg = work.tile([128, 2048], BF16, tag="wmsg")
                for c2 in range(4):
                    fs_ps = ps_fs.tile([128, 512], F32, tag="fs")
                    for j in range(2):
                        c = 2 * c2 + j
                        nc.tensor.matmul(
                            fs_ps[:, 256 * j:256 * (j + 1)],
                            bl[:, MS_OFF + 128 * c:MS_OFF + 128 * (c + 1)],
                            feat_bf[:],
                            start=True, stop=True,
                        )
                    a_in = bass.AP(
                        tensor=a_er[:].tensor,
                        offset=a_er[:].offset + 16 * c2 * a_er[:].ap[1][0],
                        ap=[a_er[:].ap[0], [1, 16], [0, 32]],
                    )
                    nc.vector.tensor_tensor(
                        out=wmsg[:, 512 * c2:512 * (c2 + 1)].rearrange(
                            "p (a c) -> p a c", a=16),
                        in0=fs_ps[:].rearrange("p (a c) -> p a c", a=16),
                        in1=a_in,
                        op=mybir.AluOpType.mult,
                    )

                # -------- rst = segsum(wmsg) + x --------
                rst_ps = ps_rst.tile([128, 256], F32, tag="rst")
                for c in range(8):
                    nc.tensor.matmul(
                        rst_ps[:],
                        bl[:, MDT_OFF + 128 * c:MDT_OFF + 128 * (c + 1)],
                        wmsg[:, 256 * c:256 * (c + 1)],
                        start=(c == 0), stop=(c == 7),
                    )
                nc.vector.tensor_tensor(
                    out=rst_all[:, F * g:F * (g + 1)], in0=rst_ps[:], in1=xg[:],
                    op=mybir.AluOpType.add,
                )

                # -------- BN stat accumulation --------
                rst_bf = small.tile([128, F], BF16, tag="rst_bf")
                nc.gpsimd.tensor_copy(rst_bf[:], rst_all[:, F * g:F * (g + 1)])
                sq_bf = small.tile([128, F], BF16, tag="sq_bf")
                nc.vector.tensor_tensor(
                    out=sq_bf[:], in0=rst_all[:, F * g:F * (g + 1)],
                    in1=rst_all[:, F * g:F * (g + 1)], op=mybir.AluOpType.mult,
                )
                nc.tensor.matmul(bn_ps[:], ones_bf[:], rst_bf[:],
                                 start=(g == 0), stop=(g == G - 1))
                nc.tensor.matmul(bnsq_ps[:], ones_bf[:], sq_bf[:],
                                 start=(g == 0), stop=(g == G - 1))

            # ---------------- BN allreduce + apply ----------------
            bn_sb = statics.tile([1, 512], F32, tag="bn_sb")
            nc.vector.tensor_copy(bn_sb[:], bnacc_ps[:])
            nc.sync.dma_start(cc_in[:], bn_sb[:])
            if debug_dump:
                nc.sync.dma_start(dbg[5, 0:1, :256], bn_sb[:, :256])
                nc.sync.dma_start(dbg[5, 1:2, :256], bn_sb[:, 256:])
            bn_red = statics.tile([1, 512], F32, tag="bn_red")
            if use_collective:
                nc.gpsimd.collective_compute(
                    "AllReduce",
                    mybir.AluOpType.add,
                    replica_groups=[list(range(NCORES))],
                    ins=[cc_in[:]],
                    outs=[cc_out[:]],
                )
                nc.sync.dma_start(bn_red[:], cc_out[:])
            else:
                nc.sync.dma_start(bn_red[:], cc_in[:])

            # mean = sum/N; var = sumsq/N - mean^2
            mean_r = statics.tile([1, F], F32, tag="mean_r")
            nc.vector.tensor_scalar_mul(mean_r[:], bn_red[:, :256],
                                        1.0 / (NTOT if use_collective else NPC))
            m2_r = statics.tile([1, F], F32, tag="m2_r")
            nc.vector.tensor_scalar_mul(m2_r[:], bn_red[:, 256:],
                                        1.0 / (NTOT if use_collective else NPC))
            msq_r = statics.tile([1, F], F32, tag="msq_r")
            nc.vector.tensor_tensor(out=msq_r[:], in0=mean_r[:], in1=mean_r[:],
                                    op=mybir.AluOpType.mult)
            var_r = statics.tile([1, F], F32, tag="var_r")
            nc.vector.tensor_tensor(out=var_r[:], in0=m2_r[:], in1=msq_r[:],
                                    op=mybir.AluOpType.subtract)
            # A = gamma * rsqrt(var+eps); Bb = beta - mean*A
            # rsqrt(var+eps) = 1/sqrt(var+eps) (Rsqrt ACT is blocked)
            vpe_r = statics.tile([1, F], F32, tag="vpe_r")
            nc.vector.tensor_scalar_add(vpe_r[:], var_r[:], BN_EPS)
            sd_r = statics.tile([1, F], F32, tag="sd_r")
            nc.scalar.activation(sd_r[:], vpe_r[:],
                                 mybir.ActivationFunctionType.Sqrt)
            rsq_r = statics.tile([1, F], F32, tag="rsq_r")
            nc.vector.reciprocal(rsq_r[:], sd_r[:])
            A_r = statics.tile([1, F], F32, tag="A_r")
            nc.vector.tensor_tensor(out=A_r[:], in0=rsq_r[:], in1=gam_t[:],
                                    op=mybir.AluOpType.mult)
            mA_r = statics.tile([1, F], F32, tag="mA_r")
            nc.vector.tensor_tensor(out=mA_r[:], in0=mean_r[:], in1=A_r[:],
                                    op=mybir.AluOpType.mult)
            Bb_r = statics.tile([1, F], F32, tag="Bb_r")
            nc.vector.tensor_tensor(out=Bb_r[:], in0=bet_t[:], in1=mA_r[:],
                                    op=mybir.AluOpType.subtract)
            nc.sync.dma_start(bn_dram[0:1, :], A_r[:])
            nc.sync.dma_start(bn_dram[1:2, :], Bb_r[:])
            A_rep = statics.tile([128, F], F32)
            nc.gpsimd.dma_start(
                out=A_rep[:],
                in_=bass.AP(tensor=bn_dram[:].tensor, offset=bn_dram[0:1, :].offset,
                            ap=[[0, 128], [1, F]]),
            )
            Bb_rep = statics.tile([128, F], F32)
            nc.gpsimd.dma_start(
                out=Bb_rep[:],
                in_=bass.AP(tensor=bn_dram[:].tensor, offset=bn_dram[1:2, :].offset,
                            ap=[[0, 128], [1, F]]),
            )

            # BN apply in blocks of 8 graphs: [128, 2048] per op.
            # A_rep/Bb_rep broadcast-read with free-step-0 over the 8 graphs.
            GB = 8
            if debug_dump:
                nc.sync.dma_start(dbg[5, 2:3, :256], A_r[:])
                nc.sync.dma_start(dbg[5, 3:4, :256], Bb_r[:])
            for g0 in range(0, G, GB):
                t1 = bnp.tile([128, GB * F], F32, tag="bnap")
                arep_in = bass.AP(
                    tensor=A_rep[:].tensor, offset=A_rep[:].offset,
                    ap=[A_rep[:].ap[0], [0, GB], [1, F]],
                )
                nc.vector.tensor_tensor(
                    out=t1[:].rearrange("p (g f) -> p g f", g=GB),
                    in0=rst_all[:, F * g0:F * (g0 + GB)].rearrange(
                        "p (g f) -> p g f", g=GB),
                    in1=arep_in,
                    op=mybir.AluOpType.mult,
                )
                t2 = bnp.tile([128, GB * F], F32, tag="bnap2")
                bbrep_in = bass.AP(
                    tensor=Bb_rep[:].tensor, offset=Bb_rep[:].offset,
                    ap=[Bb_rep[:].ap[0], [0, GB], [1, F]],
                )
                nc.vector.tensor_tensor(
                    out=t2[:].rearrange("p (g f) -> p g f", g=GB),
                    in0=t1[:].rearrange("p (g f) -> p g f", g=GB),
                    in1=bbrep_in,
                    op=mybir.AluOpType.add,
                )
                t3 = bnp.tile([128, GB * F], F32, tag="bnap")
                nc.scalar.activation(t3[:], t2[:],
                                     mybir.ActivationFunctionType.Relu)
                out_dst = bass.AP(
                    tensor=out[:].tensor,
                    offset=out[g0].offset,
                    ap=[[F, 128], [128 * F, GB], [1, F]],
                )
                out_src = bass.AP(
                    tensor=t3[:].tensor, offset=t3[:].offset,
                    ap=[t3[:].ap[0], [F, GB], [1, F]],
                )
                nc.sync.dma_start(out_dst, out_src)

    nc.compile()
    return nc


def _host_prep(x, fc_w, fc_b, attn_w, gamma, beta, src, dst):
    """Build per-core input maps (index preprocessing + layout only)."""
    bf16 = ml_dtypes.bfloat16
    n_ids = np.arange(NODE, dtype=np.int32)

    src_l = (src.reshape(B, EPG) & (NODE - 1)).astype(np.int32)
    dst_l = (dst.reshape(B, EPG) & (NODE - 1)).astype(np.int32)

    # one-hots for all graphs at once
    oh_src = (src_l[:, None, :] == n_ids[None, :, None])  # [B, 128n, 1024e]
    oh_dst = (dst_l[:, None, :] == n_ids[None, :, None])
    asum = oh_src.astype(np.float32) + oh_dst.astype(np.float32)

    # MdstT: [e,n] chunked into [128, 8*128] per graph
    mdstT = np.transpose(oh_dst, (0, 2, 1)).reshape(B, 8, 128, 128)
    mdstT = np.transpose(mdstT, (0, 2, 1, 3)).reshape(B, 128, 1024)

    xT = x.reshape(B, NODE, IN).transpose(0, 2, 1)          # [B, 256in, 128n]
    xT = xT.reshape(B, 2, 128, 128).transpose(0, 2, 1, 3).reshape(B, 128, 256)

    blob = np.empty((B, 128, BLOB_COLS), dtype=bf16)
    blob[:, :, A_OFF:A_OFF + 1024] = asum.astype(bf16)
    blob[:, :, MS_OFF:MS_OFF + 1024] = oh_src.astype(bf16)
    blob[:, :, MDT_OFF:MDT_OFF + 1024] = mdstT.astype(bf16)
    blob[:, :, MD_OFF:MD_OFF + 1024] = oh_dst.astype(bf16)
    blob[:, :, XT_OFF:XT_OFF + 256] = xT.astype(bf16)

    fcwT = fc_w.T.reshape(2, 128, F).transpose(1, 0, 2).astype(bf16)
    wbm = np.zeros((F, H), dtype=np.float32)                # block-diag attn_w
    for h in range(H):
        wbm[h * D:(h + 1) * D, h] = attn_w[h]
    wb = wbm.reshape(2, 128, H).transpose(1, 0, 2).astype(bf16)
    wcomb_m = NEG_SLOPE * (fc_w.T @ wbm)                    # [256in, 8]
    wcomb = wcomb_m.reshape(2, 128, H).transpose(1, 0, 2).astype(bf16)
    dwb = (NEG_SLOPE * (fc_b @ wbm)).reshape(1, H).astype(bf16)

    xg = x.reshape(B, NODE, IN).astype(np.float32)

    in_maps = []
    for c in range(NCORES):
        sl = slice(c * G, (c + 1) * G)
        in_maps.append({
            "blob": np.ascontiguousarray(blob[sl]),
            "xf": np.ascontiguousarray(xg[sl]),
            "fcwT": fcwT,
            "fcb": fc_b.reshape(1, F).astype(np.float32),
            "wb": wb,
            "wcomb": wcomb,
            "dwb": dwb,
            "gam": gamma.reshape(1, F).astype(np.float32),
            "bet": beta.reshape(1, F).astype(np.float32),
        })
    return in_maps


_CACHED = {}


def kernel(x, fc_w, fc_b, attn_w, gamma, beta, src, dst, batch_size, **run_kw):
    x = np.asarray(x, np.float32)
    fc_w = np.asarray(fc_w, np.float32)
    fc_b = np.asarray(fc_b, np.float32)
    attn_w = np.asarray(attn_w, np.float32)
    gamma = np.asarray(gamma, np.float32)
    beta = np.asarray(beta, np.float32)
    src = np.asarray(src, np.int32)
    dst = np.asarray(dst, np.int32)

    in_maps = _host_prep(x, fc_w, fc_b, attn_w, gamma, beta, src, dst)
    if "nc" not in _CACHED:
        _CACHED["nc"] = build_bass()
    nc = _CACHED["nc"]

    res = run_bass_kernel_spmd(nc, in_maps, core_ids=list(range(NCORES)), **run_kw)
    outs = res.results

    out = np.concatenate([r["out"].reshape(NPC, F) for r in outs], axis=0)
    # attn computed edge-major [G, EPG, H]; reorder to [B, H, EPG] (host-side
    # layout move during unshard)
    attn_full = np.concatenate(
        [r["attn"].transpose(0, 2, 1, 3).reshape(G, EPG, H) for r in outs],
        axis=0)
    attn_full = np.ascontiguousarray(attn_full.transpose(0, 2, 1))
    if run_kw:
        return (out, attn_full), res
    return out, attn_full


if __name__ == "__main__":
    # quick static check: no Matmult with >1 sem wait
    nc = build_bass()
    bad = 0
    n_mm = 0
    for f in nc.m.functions:
        for blk in f.blocks:
            for ins in blk.instructions:
                if type(ins).__name__ == "InstMatmult":
                    n_mm += 1
                    w = ins.sync_info.on_wait if ins.sync_info else []
                    if len(w) > 1:
                        bad += 1
                        if bad <= 10:
                            print("MULTI-WAIT", ins.name, w)
    print(f"matmults: {n_mm}, multi-wait: {bad}")


# revision 35
# speedup vs baseline: 1.0172x; 1.0172x over previous
"""GATv2-style DeepGraphConvLayer on 8 Trainium2 NeuronCores.

Strategy (graph-data-parallel, per sharding hint):
  - 8 cores x 64 graphs each; each graph = 128 nodes / 1024 edges.
  - All gather/scatter (segment ops) done as PE matmuls against host-built
    one-hot matrices (bf16, exact 0/1/2 values):
      * ehT   = feat.T-gathered-sum   via lhsT=feat chunk, rhs=Asum[n,e]
      * fs    = feat[src] (edge-row)  via lhsT=Msrc[n,e] chunk, rhs=feat
      * s     = segsum(expl) by dst   via lhsT=MdstT[e,n] chunk, rhs=expl_er
      * rs_er = recip_s[dst] gather   via lhsT=Mdst[n,e] chunk, rhs=recip_s
      * rst   = segsum(wmsg) by dst   via lhsT=MdstT[e,n] chunk, rhs=wmsg
  - leaky_relu fused into the PSUM->SBUF evacuation (ACT Lrelu).
  - softmax denominators via segment-sum of exp (no max subtraction: logits
    are O(1); shift cancels mathematically anyway).
  - messages weighted by normalized a at the edge level; BatchNorm batch
  - stats all-reduced across the 8 cores with a DRAM collective.
Outputs: out [N,256] f32, attn written edge-major [g,1024,8] and transposed
to [B,H,EPG] on the host during unshard (pure data movement).
"""
import numpy as np
import ml_dtypes
import concourse.bass as bass
import concourse.bacc as bacc
import concourse.tile as tile
from concourse import mybir
from concourse.bass_utils import run_bass_kernel_spmd
from concourse.masks import make_identity

F32 = mybir.dt.float32
BF16 = mybir.dt.bfloat16

NCORES = 8
B = 512
NODE = 128
EPG = 1024
H = 8
D = 32
IN = 256
F = 256           # H*D
G = B // NCORES   # 64 graphs per core
NPC = G * NODE    # 8192 nodes per core
EPC = G * EPG     # 65536 edges per core
NEG_SLOPE = 0.2
BN_EPS = 1e-5
NTOT = B * NODE   # 65536

# bf16 blob layout (per graph), columns along free dim:
#   Asum  [128, 1024]  cols 0:1024      (n-part, e)
#   Msrc  [128, 1024]  cols 1024:2048   (n-part, e)  chunks: [:, 128c:128c+128]
#   MdstT [128, 1024]  cols 2048:3072   (e-part chunks, n): chunk c at [:, 128c:]
#   Mdst  [128, 1024]  cols 3072:4096   (n-part, e)
#   xT    [128, 256]   cols 4096:4352   (in-part chunks, n): chunk k at [:, 128k:]
BLOB_COLS = 4352
A_OFF, MS_OFF, MDT_OFF, MD_OFF, XT_OFF = 0, 1024, 2048, 3072, 4096


def build_bass(use_collective=True, debug_dump=False):
    nc = bacc.Bacc(None, target_bir_lowering=False, debug=False)

    blob = nc.dram_tensor("blob", [G, 128, BLOB_COLS], BF16, kind="ExternalInput")
    xf = nc.dram_tensor("xf", [G, 128, F], F32, kind="ExternalInput")
    fcwT = nc.dram_tensor("fcwT", [128, 2, F], BF16, kind="ExternalInput")
    fcb = nc.dram_tensor("fcb", [1, F], F32, kind="ExternalInput")
    wb = nc.dram_tensor("wb", [128, 2, H], BF16, kind="ExternalInput")
    wcomb = nc.dram_tensor("wcomb", [128, 2, H], BF16, kind="ExternalInput")
    dwb = nc.dram_tensor("dwb", [1, H], BF16, kind="ExternalInput")
    gam = nc.dram_tensor("gam", [1, F], F32, kind="ExternalInput")
    bet = nc.dram_tensor("bet", [1, F], F32, kind="ExternalInput")

    out = nc.dram_tensor("out", [G, 128, F], F32, kind="ExternalOutput")
    attn = nc.dram_tensor("attn", [G, 128, 8, H], F32, kind="ExternalOutput")

    dbg = None
    if debug_dump:
        dbg = nc.dram_tensor("dbg", [6, 128, 1024], F32, kind="ExternalOutput")
    cc_in = nc.dram_tensor("cc_in", [1, 512], F32)
    bn_dram = nc.dram_tensor("bn_dram", [2, F], F32)
    cc_out = nc.dram_tensor("cc_out", [1, 512], F32)

    with nc.allow_low_precision(reason="bf16 edge pipeline by design"), \
         tile.TileContext(nc) as tc:
        with (
            tc.tile_pool(name="statics", bufs=1) as statics,
            tc.tile_pool(name="rstbuf", bufs=1) as rstbuf,
            tc.tile_pool(name="blobs", bufs=4) as blobs,
            tc.tile_pool(name="work", bufs=3) as work,
            tc.tile_pool(name="bnp", bufs=2) as bnp,
            tc.tile_pool(name="small", bufs=3) as small,
            tc.tile_pool(name="ps_big", bufs=2, space="PSUM") as ps_big,
            tc.tile_pool(name="ps_log", bufs=2, space="PSUM") as ps_log,
            tc.tile_pool(name="ps_small", bufs=1, space="PSUM") as ps_small,
            tc.tile_pool(name="ps_fs", bufs=1, space="PSUM") as ps_fs,
            tc.tile_pool(name="ps_rst", bufs=1, space="PSUM") as ps_rst,
            tc.tile_pool(name="ps_bn", bufs=1, space="PSUM") as ps_bn,
        ):
            # ---------------- statics ----------------
            ident = statics.tile([128, 128], F32)
            make_identity(nc, ident[:])
            ones_bf = statics.tile([128, 1], BF16)
            nc.vector.memset(ones_bf[:], 1.0)
            ones_row = statics.tile([1, 128], BF16)
            nc.vector.memset(ones_row[:], 1.0)
            fcwT_t = statics.tile([128, 2, F], BF16)
            nc.gpsimd.dma_start(fcwT_t[:], fcwT[:])
            wb_t = statics.tile([128, 2, H], BF16)
            nc.gpsimd.dma_start(wb_t[:], wb[:])
            wcomb_t = statics.tile([128, 2, H], BF16)
            nc.gpsimd.dma_start(wcomb_t[:], wcomb[:])
            dwb_t = statics.tile([1, H], BF16)
            nc.gpsimd.dma_start(dwb_t[:], dwb[:])
            fcb_t = statics.tile([1, F], F32)
            nc.gpsimd.dma_start(fcb_t[:], fcb[:])
            gam_t = statics.tile([1, F], F32)
            nc.gpsimd.dma_start(gam_t[:], gam[:])
            bet_t = statics.tile([1, F], F32)
            nc.gpsimd.dma_start(bet_t[:], bet[:])
            fcb_rep = statics.tile([128, F], F32)
            nc.gpsimd.dma_start(
                out=fcb_rep[:],
                in_=bass.AP(tensor=fcb[:].tensor, offset=fcb[:].offset,
                            ap=[[0, 128], [1, F]]),
            )

            # rst for all 64 graphs stays in SBUF: [128, 64*256] f32
            rst_all = rstbuf.tile([128, G * F], F32)

            bnacc_ps = ps_bn.tile([1, 512], F32)
            bn_ps = bnacc_ps[:, :256]
            bnsq_ps = bnacc_ps[:, 256:]

            # "touch" matmuls so statics' DMA waits are absorbed by PE before
            # the real matmuls (Matmult carries at most ONE sem wait).
            nc.tensor.matmul(bn_ps[:1, :1], fcwT_t[:1, 0, :1],
                             fcwT_t[:1, 0, :1], start=True, stop=True)
            nc.tensor.matmul(bn_ps[:1, :1], wb_t[:1, 0, :1],
                             wb_t[:1, 0, :1], start=True, stop=True)
            nc.tensor.matmul(bn_ps[:1, :1], ones_bf[:1, :1],
                             ones_bf[:1, :1], start=True, stop=True)
            nc.tensor.matmul(bn_ps[:1, :1], wcomb_t[:1, 0, :1],
                             wcomb_t[:1, 0, :1], start=True, stop=True)
            nc.tensor.matmul(bn_ps[:1, :1], ones_row[:1, :1],
                             dwb_t[:1, :1], start=True, stop=True)
            nc.tensor.transpose(bn_ps[:1, :1], ident[:1, :1], ident[:1, :1])

            for g in range(G):
                # -------- per-graph loads (2 DMAs) --------
                bl = blobs.tile([128, BLOB_COLS], BF16, tag="bl")
                nc.sync.dma_start(bl[:], blob[g])
                xg = blobs.tile([128, F], F32, tag="xg")
                nc.sync.dma_start(xg[:], xf[g])

                # -------- fc: feat = x @ fc_w.T + b --------
                feat_ps = ps_big.tile([128, 512], F32, tag="big")
                for k in range(2):
                    nc.tensor.matmul(
                        feat_ps[:, :F],
                        bl[:, XT_OFF + 128 * k:XT_OFF + 128 * (k + 1)],
                        fcwT_t[:, k, :],
                        start=(k == 0), stop=(k == 1),
                    )
                for k in range(2):
                    nc.tensor.matmul(
                        feat_ps[:, 256:256 + H],
                        bl[:, XT_OFF + 128 * k:XT_OFF + 128 * (k + 1)],
                        wcomb_t[:, k, :],
                        start=(k == 0), stop=False,
                    )
                nc.tensor.matmul(
                    feat_ps[:, 256:256 + H], ones_row[:], dwb_t[:],
                    start=False, stop=True,
                )
                feat_bf = work.tile([128, F], BF16, tag="feat")
                nc.vector.tensor_tensor(
                    out=feat_bf[:], in0=feat_ps[:, :F], in1=fcb_rep[:],
                    op=mybir.AluOpType.add,
                )
                dw_bf = small.tile([128, H], BF16, tag="dw_bf")
                nc.vector.tensor_copy(dw_bf[:], feat_ps[:, 256:256 + H])

                # -------- ehT + leaky (2 fchunks x 2 ehalves) --------
                lky0 = work.tile([128, 1024], BF16, tag="lky0")
                lky1 = work.tile([128, 1024], BF16, tag="lky1")
                lky = [lky0, lky1]
                for k in range(2):
                    for h2 in range(2):
                        eh_ps = ps_big.tile([128, 512], F32, tag="big")
                        nc.tensor.matmul(
                            eh_ps[:],
                            feat_bf[:, 128 * k:128 * (k + 1)],
                            bl[:, A_OFF + 512 * h2:A_OFF + 512 * (h2 + 1)],
                            start=True, stop=True,
                        )
                        nc.scalar.activation(
                            lky[k][:, 512 * h2:512 * (h2 + 1)], eh_ps[:],
                            mybir.ActivationFunctionType.Relu, scale=0.8,
                        )

                if debug_dump and g == 0:
                    dfeat = work.tile([128, F], F32, tag="dfeat")
                    nc.vector.tensor_copy(dfeat[:], feat_bf[:])
                    nc.sync.dma_start(dbg[0, :, :F], dfeat[:])
                    dlky = work.tile([128, 1024], F32, tag="dlky")
                    nc.vector.tensor_copy(dlky[:], lky[0][:])
                    nc.sync.dma_start(dbg[1], dlky[:])

                # -------- logits + exp --------
                expl_T = small.tile([8, 1024], F32, tag="explT")
                for h2 in range(2):
                    log_ps = ps_log.tile([8, 512], F32, tag="log")
                    for k in range(2):
                        nc.tensor.matmul(
                            log_ps[:],
                            wb_t[:, k, :],
                            lky[k][:, 512 * h2:512 * (h2 + 1)],
                            start=(k == 0), stop=False,
                        )
                    nc.tensor.matmul(
                        log_ps[:],
                        dw_bf[:],
                        bl[:, A_OFF + 512 * h2:A_OFF + 512 * (h2 + 1)],
                        start=False, stop=True,
                    )
                    nc.scalar.activation(
                        expl_T[:, 512 * h2:512 * (h2 + 1)], log_ps[:],
                        mybir.ActivationFunctionType.Exp,
                    )

                if debug_dump and g == 0:
                    nc.sync.dma_start(dbg[2, :8, :], expl_T[:])

                # -------- expl_er: [8,1024] -> edge-row [128, 8c+h] --------
                er_ps = ps_small.tile([128, 512], F32, tag="small")
                for c in range(8):
                    nc.tensor.transpose(
                        er_ps[:, 8 * c:8 * (c + 1)],
                        expl_T[:, 128 * c:128 * (c + 1)],
                        ident[:8, :8],
                    )
                expl_er = small.tile([128, 64], F32, tag="expl_er")
                nc.vector.tensor_copy(expl_er[:], er_ps[:, :64])
                expl_er_bf = small.tile([128, 64], BF16, tag="expl_er_bf")
                nc.vector.tensor_copy(expl_er_bf[:], expl_er[:])

                # -------- s = segsum(expl) by dst; recip --------
                for c in range(8):
                    nc.tensor.matmul(
                        er_ps[:, 64:64 + 8],
                        bl[:, MDT_OFF + 128 * c:MDT_OFF + 128 * (c + 1)],
                        expl_er_bf[:, 8 * c:8 * (c + 1)],
                        start=(c == 0), stop=(c == 7),
                    )
                # clamp: isolated nodes have s=0; recip(0)=inf would
                # poison the gather matmul (0*inf=NaN)
                s_cl = small.tile([128, 8], F32, tag="s_cl")
                nc.vector.tensor_scalar_max(s_cl[:], er_ps[:, 64:64 + 8], 1e-12)
                recip_bf = small.tile([128, 8], BF16, tag="recip")
                nc.vector.reciprocal(recip_bf[:], s_cl[:])

                # -------- rs_er = recip_s[dst] gather (edge-row) --------
                for c in range(8):
                    nc.tensor.matmul(
                        er_ps[:, 128 + 8 * c:128 + 8 * (c + 1)],
                        bl[:, MD_OFF + 128 * c:MD_OFF + 128 * (c + 1)],
                        recip_bf[:],
                        start=True, stop=True,
                    )
                if debug_dump and g == 0:
                    dser = work.tile([128, 192], F32, tag="dser")
                    nc.vector.tensor_copy(dser[:], er_ps[:, :192])
                    nc.sync.dma_start(dbg[3, :, :192], dser[:])

                a_er = small.tile([128, 64], F32, tag="a_er")
                nc.vector.tensor_tensor(
                    out=a_er[:], in0=expl_er[:], in1=er_ps[:, 128:128 + 64],
                    op=mybir.AluOpType.mult,
                )
                # attn out in a_er layout [128 e', (c, h)]; host reorders
                nc.sync.dma_start(attn[g], a_er[:].rearrange(
                    "p (c h) -> p c h", c=8))

                if debug_dump and g == 0:
                    nc.sync.dma_start(dbg[4, :, :64], a_er[:])

                # -------- fs + wmsg (2 chunks per psum tile) --------
                wmsg = work.tile([128, 2048], BF16, tag="wmsg")
                for c2 in range(4):
                    fs_ps = ps_fs.tile([128, 512], F32, tag="fs")
                    for j in range(2):
                        c = 2 * c2 + j
                        nc.tensor.matmul(
                            fs_ps[:, 256 * j:256 * (j + 1)],
                            bl[:, MS_OFF + 128 * c:MS_OFF + 128 * (c + 1)],
                            feat_bf[:],
                            start=True, stop=True,
                        )
                    a_in = bass.AP(
                        tensor=a_er[:].tensor,
                        offset=a_er[:].offset + 16 * c2 * a_er[:].ap[1][0],
                        ap=[a_er[:].ap[0], [1, 16], [0, 32]],
                    )
                    nc.vector.tensor_tensor(
                        out=wmsg[:, 512 * c2:512 * (c2 + 1)].rearrange(
                            "p (a c) -> p a c", a=16),
                        in0=fs_ps[:].rearrange("p (a c) -> p a c", a=16),
                        in1=a_in,
                        op=mybir.AluOpType.mult,
                    )

                # -------- rst = segsum(wmsg) + x --------
                rst_ps = ps_rst.tile([128, 256], F32, tag="rst")
                for c in range(8):
                    nc.tensor.matmul(
                        rst_ps[:],
                        bl[:, MDT_OFF + 128 * c:MDT_OFF + 128 * (c + 1)],
                        wmsg[:, 256 * c:256 * (c + 1)],
                        start=(c == 0), stop=(c == 7),
                    )
                nc.vector.tensor_tensor(
                    out=rst_all[:, F * g:F * (g + 1)], in0=rst_ps[:], in1=xg[:],
                    op=mybir.AluOpType.add,
                )

                # -------- BN stat accumulation --------
                rst_bf = small.tile([128, F], BF16, tag="rst_bf")
                nc.gpsimd.tensor_copy(rst_bf[:], rst_all[:, F * g:F * (g + 1)])
                sq_bf = small.tile([128, F], BF16, tag="sq_bf")
                nc.vector.tensor_tensor(
                    out=sq_bf[:], in0=rst_all[:, F * g:F * (g + 1)],
                    in1=rst_all[:, F * g:F * (g + 1)], op=mybir.AluOpType.mult,
                )
                nc.tensor.matmul(bn_ps[:], ones_bf[:], rst_bf[:],
                                 start=(g == 0), stop=(g == G - 1))
                nc.tensor.matmul(bnsq_ps[:], ones_bf[:], sq_bf[:],
                                 start=(g == 0), stop=(g == G - 1))

            # ---------------- BN allreduce + apply ----------------
            bn_sb = statics.tile([1, 512], F32, tag="bn_sb")
            nc.vector.tensor_copy(bn_sb[:], bnacc_ps[:])
            nc.sync.dma_start(cc_in[:], bn_sb[:])
            if debug_dump:
                nc.sync.dma_start(dbg[5, 0:1, :256], bn_sb[:, :256])
                nc.sync.dma_start(dbg[5, 1:2, :256], bn_sb[:, 256:])
            bn_red = statics.tile([1, 512], F32, tag="bn_red")
            if use_collective:
                nc.gpsimd.collective_compute(
                    "AllReduce",
                    mybir.AluOpType.add,
                    replica_groups=[list(range(NCORES))],
                    ins=[cc_in[:]],
                    outs=[cc_out[:]],
                )
                nc.sync.dma_start(bn_red[:], cc_out[:])
            else:
                nc.sync.dma_start(bn_red[:], cc_in[:])

            # mean = sum/N; var = sumsq/N - mean^2
            mean_r = statics.tile([1, F], F32, tag="mean_r")
            nc.vector.tensor_scalar_mul(mean_r[:], bn_red[:, :256],
                                        1.0 / (NTOT if use_collective else NPC))
            m2_r = statics.tile([1, F], F32, tag="m2_r")
            nc.vector.tensor_scalar_mul(m2_r[:], bn_red[:, 256:],
                                        1.0 / (NTOT if use_collective else NPC))
            msq_r = statics.tile([1, F], F32, tag="msq_r")
            nc.vector.tensor_tensor(out=msq_r[:], in0=mean_r[:], in1=mean_r[:],
                                    op=mybir.AluOpType.mult)
            var_r = statics.tile([1, F], F32, tag="var_r")
            nc.vector.tensor_tensor(out=var_r[:], in0=m2_r[:], in1=msq_r[:],
                                    op=mybir.AluOpType.subtract)
            # A = gamma * rsqrt(var+eps); Bb = beta - mean*A
            # rsqrt(var+eps) = 1/sqrt(var+eps) (Rsqrt ACT is blocked)
            vpe_r = statics.tile([1, F], F32, tag="vpe_r")
            nc.vector.tensor_scalar_add(vpe_r[:], var_r[:], BN_EPS)
            sd_r = statics.tile([1, F], F32, tag="sd_r")
            nc.scalar.activation(sd_r[:], vpe_r[:],
                                 mybir.ActivationFunctionType.Sqrt)
            rsq_r = statics.tile([1, F], F32, tag="rsq_r")
            nc.vector.reciprocal(rsq_r[:], sd_r[:])
            A_r = statics.tile([1, F], F32, tag="A_r")
            nc.vector.tensor_tensor(out=A_r[:], in0=rsq_r[:], in1=gam_t[:],
                                    op=mybir.AluOpType.mult)
            mA_r = statics.tile([1, F], F32, tag="mA_r")
            nc.vector.tensor_tensor(out=mA_r[:], in0=mean_r[:], in1=A_r[:],
                                    op=mybir.AluOpType.mult)
            Bb_r = statics.tile([1, F], F32, tag="Bb_r")
            nc.vector.tensor_tensor(out=Bb_r[:], in0=bet_t[:], in1=mA_r[:],
                                    op=mybir.AluOpType.subtract)
            nc.sync.dma_start(bn_dram[0:1, :], A_r[:])
            nc.sync.dma_start(bn_dram[1:2, :], Bb_r[:])
            A_rep = statics.tile([128, F], F32)
            nc.gpsimd.dma_start(
                out=A_rep[:],
                in_=bass.AP(tensor=bn_dram[:].tensor, offset=bn_dram[0:1, :].offset,
                            ap=[[0, 128], [1, F]]),
            )
            Bb_rep = statics.tile([128, F], F32)
            nc.gpsimd.dma_start(
                out=Bb_rep[:],
                in_=bass.AP(tensor=bn_dram[:].tensor, offset=bn_dram[1:2, :].offset,
                            ap=[[0, 128], [1, F]]),
            )

            # BN apply in blocks of 8 graphs: [128, 2048] per op.
            # A_rep/Bb_rep broadcast-read with free-step-0 over the 8 graphs.
            GB = 8
            if debug_dump:
                nc.sync.dma_start(dbg[5, 2:3, :256], A_r[:])
                nc.sync.dma_start(dbg[5, 3:4, :256], Bb_r[:])
            for g0 in range(0, G, GB):
                t1 = bnp.tile([128, GB * F], F32, tag="bnap")
                arep_in = bass.AP(
                    tensor=A_rep[:].tensor, offset=A_rep[:].offset,
                    ap=[A_rep[:].ap[0], [0, GB], [1, F]],
                )
                nc.vector.tensor_tensor(
                    out=t1[:].rearrange("p (g f) -> p g f", g=GB),
                    in0=rst_all[:, F * g0:F * (g0 + GB)].rearrange(
                        "p (g f) -> p g f", g=GB),
                    in1=arep_in,
                    op=mybir.AluOpType.mult,
                )
                t2 = bnp.tile([128, GB * F], F32, tag="bnap2")
                bbrep_in = bass.AP(
                    tensor=Bb_rep[:].tensor, offset=Bb_rep[:].offset,
                    ap=[Bb_rep[:].ap[0], [0, GB], [1, F]],
                )
                nc.vector.tensor_tensor(
                    out=t2[:].rearrange("p (g f) -> p g f", g=GB),
                    in0=t1[:].rearrange("p (g f) -> p g f", g=GB),
                    in1=bbrep_in,
                    op=mybir.AluOpType.add,
                )
                t3 = bnp.tile([128, GB * F], F32, tag="bnap")
                nc.scalar.activation(t3[:], t2[:],
                                     mybir.ActivationFunctionType.Relu)
                out_dst = bass.AP(
                    tensor=out[:].tensor,
                    offset=out[g0].offset,
                    ap=[[F, 128], [128 * F, GB], [1, F]],
                )
                out_src = bass.AP(
                    tensor=t3[:].tensor, offset=t3[:].offset,
                    ap=[t3[:].ap[0], [F, GB], [1, F]],
                )
                nc.sync.dma_start(out_dst, out_src)

    nc.compile()
    return nc


def _host_prep(x, fc_w, fc_b, attn_w, gamma, beta, src, dst):
    """Build per-core input maps (index preprocessing + layout only)."""
    bf16 = ml_dtypes.bfloat16
    n_ids = np.arange(NODE, dtype=np.int32)

    src_l = (src.reshape(B, EPG) & (NODE - 1)).astype(np.int32)
    dst_l = (dst.reshape(B, EPG) & (NODE - 1)).astype(np.int32)

    # one-hots for all graphs at once
    oh_src = (src_l[:, None, :] == n_ids[None, :, None])  # [B, 128n, 1024e]
    oh_dst = (dst_l[:, None, :] == n_ids[None, :, None])
    asum = oh_src.astype(np.float32) + oh_dst.astype(np.float32)

    # MdstT: [e,n] chunked into [128, 8*128] per graph
    mdstT = np.transpose(oh_dst, (0, 2, 1)).reshape(B, 8, 128, 128)
    mdstT = np.transpose(mdstT, (0, 2, 1, 3)).reshape(B, 128, 1024)

    xT = x.reshape(B, NODE, IN).transpose(0, 2, 1)          # [B, 256in, 128n]
    xT = xT.reshape(B, 2, 128, 128).transpose(0, 2, 1, 3).reshape(B, 128, 256)

    blob = np.empty((B, 128, BLOB_COLS), dtype=bf16)
    blob[:, :, A_OFF:A_OFF + 1024] = asum.astype(bf16)
    blob[:, :, MS_OFF:MS_OFF + 1024] = oh_src.astype(bf16)
    blob[:, :, MDT_OFF:MDT_OFF + 1024] = mdstT.astype(bf16)
    blob[:, :, MD_OFF:MD_OFF + 1024] = oh_dst.astype(bf16)
    blob[:, :, XT_OFF:XT_OFF + 256] = xT.astype(bf16)

    fcwT = fc_w.T.reshape(2, 128, F).transpose(1, 0, 2).astype(bf16)
    wbm = np.zeros((F, H), dtype=np.float32)                # block-diag attn_w
    for h in range(H):
        wbm[h * D:(h + 1) * D, h] = attn_w[h]
    wb = wbm.reshape(2, 128, H).transpose(1, 0, 2).astype(bf16)
    wcomb_m = NEG_SLOPE * (fc_w.T @ wbm)                    # [256in, 8]
    wcomb = wcomb_m.reshape(2, 128, H).transpose(1, 0, 2).astype(bf16)
    dwb = (NEG_SLOPE * (fc_b @ wbm)).reshape(1, H).astype(bf16)

    xg = x.reshape(B, NODE, IN).astype(np.float32)

    in_maps = []
    for c in range(NCORES):
        sl = slice(c * G, (c + 1) * G)
        in_maps.append({
            "blob": np.ascontiguousarray(blob[sl]),
            "xf": np.ascontiguousarray(xg[sl]),
            "fcwT": fcwT,
            "fcb": fc_b.reshape(1, F).astype(np.float32),
            "wb": wb,
            "wcomb": wcomb,
            "dwb": dwb,
            "gam": gamma.reshape(1, F).astype(np.float32),
            "bet": beta.reshape(1, F).astype(np.float32),
        })
    return in_maps


_CACHED = {}


def kernel(x, fc_w, fc_b, attn_w, gamma, beta, src, dst, batch_size, **run_kw):
    x = np.asarray(x, np.float32)
    fc_w = np.asarray(fc_w, np.float32)
    fc_b = np.asarray(fc_b, np.float32)
    attn_w = np.asarray(attn_w, np.float32)
    gamma = np.asarray(gamma, np.float32)
    beta = np.asarray(beta, np.float32)
    src = np.asarray(src, np.int32)
    dst = np.asarray(dst, np.int32)

    in_maps = _host_prep(x, fc_w, fc_b, attn_w, gamma, beta, src, dst)
    if "nc" not in _CACHED:
        _CACHED["nc"] = build_bass()
    nc = _CACHED["nc"]

    res = run_bass_kernel_spmd(nc, in_maps, core_ids=list(range(NCORES)), **run_kw)
    outs = res.results

    out = np.concatenate([r["out"].reshape(NPC, F) for r in outs], axis=0)
    # attn computed edge-major [G, EPG, H]; reorder to [B, H, EPG] (host-side
    # layout move during unshard)
    attn_full = np.concatenate(
        [r["attn"].transpose(0, 2, 1, 3).reshape(G, EPG, H) for r in outs],
        axis=0)
    attn_full = np.ascontiguousarray(attn_full.transpose(0, 2, 1))
    if run_kw:
        return (out, attn_full), res
    return out, attn_full


if __name__ == "__main__":
    # quick static check: no Matmult with >1 sem wait
    nc = build_bass()
    bad = 0
    n_mm = 0
    for f in nc.m.functions:
        for blk in f.blocks:
            for ins in blk.instructions:
                if type(ins).__name__ == "InstMatmult":
                    n_mm += 1
                    w = ins.sync_info.on_wait if ins.sync_info else []
                    if len(w) > 1:
                        bad += 1
                        if bad <= 10:
                            print("MULTI-WAIT", ins.name, w)
    print(f"matmults: {n_mm}, multi-wait: {bad}")
